# revision 23
# baseline (speedup 1.0000x reference)
"""Bidirectional LSTM layer on 8 TRN2 NeuronCores.

Problem: T=512, B=64, I=H=512.  out = concat(LSTM_fwd(x), LSTM_bwd(x)).

Sharding: the recurrence streaming cost on the TensorEngine is independent
of batch size (the moving operand is the weight matrix), so batch sharding
alone does not help, and per-step collectives (AllGather floor ~4.6us) are
far too slow for 512 sequential steps.  Design:

  core 0: fwd direction, batch rows  0:32      core 2: bwd, batch  0:32
  core 1: fwd direction, batch rows 32:64      core 3: bwd, batch 32:64
  cores 4-7: same SPMD program on duplicate data (outputs ignored).

Per core the gate pre-activations are computed in a partition-packed
layout: PSUM [128, 512] where partitions 32q:32q+32 hold batch rows for
weight-column quarter q (columns permuted so quarter q = [i|f|o|g] of
hidden slice 128q:128q+128).  This packs sigmoid/tanh into [128, F]
ScalarE calls and the cell update into [128, 128] VectorE calls.

The input projection (x @ W_ih + b, no recurrence dependency) is computed
in blocks of 4 timesteps with a full-width M=128 stationary operand
(4 timesteps x 32 batch), amortizing it to 5 matmul slots per step; it is
injected into the packed gates via identity matmuls from partition rows
32t.  The h @ W_hh part runs per step (16 matmuls of N=512).  h is
transposed back to [H, B] layout for the next step's stationary operand
with a single full 128x128 TensorE transpose.  Matmul operands are fp16
(PSUM accumulation and all state/activations stay fp32); measured
end-to-end relative error vs the fp32 reference is ~3.7e-4 at T=512.

Measured on silicon (For_i differential probe, constant NEFF + host I/O):
~4.9 us/step -> ~2.5 ms for the full 512-step bidirectional pass.  The
step time is bound by the TensorEngine instruction stream (25 N=512
matmul slots/step at ~180-200 ns each; matmuls issue in-order and column
-tiled streams only partially overlap on this hardware).
"""

import os
import numpy as np

import concourse.bass as bass
import concourse.tile as tile
from concourse import bacc, mybir
from concourse.bass_utils import run_bass_kernel_spmd

# problem dims (hardcoded per the task)
T, B, I, H = 512, 64, 512, 512
G4 = 4 * H

F32 = mybir.dt.float32
F16 = mybir.dt.float16
MM_DTYPE = F16            # matmul operand dtype (PSUM accumulation stays f32)
MM_NP = np.float16

NQ = 4                      # partition-stacked quarter blocks = 128 // B_CORE
B_CORE = 128 // NQ          # batch rows per core
WQ = G4 // NQ               # weight cols per quarter stream
HQ = WQ // 4                # hidden units per quarter block
KC = I // 128               # x K-chunks (4)
KH = H // 128               # h K-chunks (4)
NTR = HQ // 128             # full-width transposes per step (1 for NQ=4)
TB = 128 // B_CORE          # timesteps per xp block (4)

_cache = {}


def _build(nt, mm_dtype=MM_DTYPE, act_dtype=F32, repeat=1):
    """Build the per-core Bacc program for an nt-step recurrence."""
    assert nt % TB == 0
    nb = nt // TB
    nc = bacc.Bacc("TRN2", target_bir_lowering=False, debug=False)

    wc = nc.dram_tensor("wc", [1024, G4], mm_dtype, kind="ExternalInput").ap()
    wb = nc.dram_tensor("wb", [1, G4], mm_dtype, kind="ExternalInput").ap()
    xblk = nc.dram_tensor("xblk", [nb, I, 128], mm_dtype, kind="ExternalInput").ap()
    h0t = nc.dram_tensor("h0t", [128, NTR, 128], mm_dtype, kind="ExternalInput").ap()
    c0p = nc.dram_tensor("c0p", [128, H // NQ], F32, kind="ExternalInput").ap()
    ident = nc.dram_tensor("ident", [128, 128], F32, kind="ExternalInput").ap()
    identq = nc.dram_tensor("identq", [128, B_CORE], mm_dtype,
                            kind="ExternalInput").ap()
    ones = nc.dram_tensor("ones", [1, 128], mm_dtype, kind="ExternalInput").ap()

    out_h = nc.dram_tensor("out_h", [nt, B_CORE, H], F32, kind="ExternalOutput").ap()
    c_fin = nc.dram_tensor("c_fin", [128, H // NQ], F32, kind="ExternalOutput").ap()

    SIG = mybir.ActivationFunctionType.Sigmoid
    TANH = mybir.ActivationFunctionType.Tanh

    with tile.TileContext(nc) as tc:
        with (
            tc.tile_pool(name="const", bufs=1) as const_pool,
            tc.tile_pool(name="xin", bufs=3) as xin_pool,
            tc.tile_pool(name="xps", bufs=2) as xps_pool,
            tc.tile_pool(name="state", bufs=2) as state_pool,
            tc.tile_pool(name="act", bufs=2) as act_pool,
            tc.tile_pool(name="pg", bufs=2, space="PSUM") as pg_pool,
            tc.tile_pool(name="pt", bufs=2, space="PSUM") as pt_pool,
            tc.tile_pool(name="pxp", bufs=1, space="PSUM") as pxp_pool,
        ):
            # resident weights [128, 8, 2048]: chunks 0..3 W_ih^T, 4..7 W_hh^T
            wc_t = const_pool.tile([128, (KC + KH), G4], mm_dtype)
            nc.sync.dma_start(out=wc_t, in_=wc.rearrange("(c p) n -> p c n", p=128))
            wb_t = const_pool.tile([1, G4], mm_dtype)
            nc.sync.dma_start(out=wb_t, in_=wb)
            id_t = const_pool.tile([128, 128], F32)
            nc.sync.dma_start(out=id_t, in_=ident)
            idq_t = const_pool.tile([128, B_CORE], mm_dtype)
            nc.sync.dma_start(out=idq_t, in_=identq)
            ones_t = const_pool.tile([1, 128], mm_dtype)
            nc.sync.dma_start(out=ones_t, in_=ones)

            # initial state
            ht = state_pool.tile([128, NTR, 128], mm_dtype, tag="ht")
            nc.sync.dma_start(out=ht, in_=h0t)
            c_prev = state_pool.tile([128, H // NQ], F32, tag="c")
            nc.sync.dma_start(out=c_prev, in_=c0p)

            def dma_xblk(b):
                t = xin_pool.tile([128, KC, 128], mm_dtype, tag="xb")
                nc.sync.dma_start(
                    out=t, in_=xblk[b].rearrange("(c p) m -> p c m", p=128))
                return t

            def new_pxp():
                return [pxp_pool.tile([128, 2 * 512], F32, tag=f"px{h}",
                                      name=f"pxp{h}")
                        for h in range(2)]

            def emit_xp_chunk(k, xb_t, pxp):
                """one K-chunk (4 MMs) of an xp block GEMM; k==KC is bias."""
                for half in range(2):
                    for n in range(2):
                        lhsT = ones_t[0:1, :] if k == KC else xb_t[:, k, :]
                        rhs = (wb_t[0:1, half * 1024 + n * 512:
                                    half * 1024 + (n + 1) * 512] if k == KC
                               else wc_t[:, k, half * 1024 + n * 512:
                                         half * 1024 + (n + 1) * 512])
                        nc.tensor.matmul(
                            pxp[half][:, n * 512:(n + 1) * 512], lhsT, rhs,
                            start=(k == 0), stop=(k == KC),
                            skip_group_check=True)

            def copy_xp(pxp):
                """PSUM xp block -> SBUF fp16 (split across DVE and ACT)."""
                xps = xps_pool.tile([128, G4], mm_dtype, tag="xps")
                nc.vector.tensor_copy(xps[:, 0:1024], pxp[0])
                nc.scalar.copy(xps[:, 1024:2048], pxp[1])
                return xps

            # prologue: xp block 0
            xb = dma_xblk(0)
            pxp = new_pxp()
            for k in range(KC + 1):
                emit_xp_chunk(k, xb, pxp)
            xps_cur = copy_xp(pxp)
            xps_nxt = None
            pxp_nxt = None
            xb_nxt = None

            total = nt * repeat
            for ss in range(total):
                s = ss % nt
                bidx = s // TB
                u = s % TB

                # stage next xp block, one K-chunk per step
                if u == 0 and bidx + 1 < nb:
                    xb_nxt = dma_xblk(bidx + 1)
                    pxp_nxt = new_pxp()
                if pxp_nxt is not None:
                    emit_xp_chunk(u, xb_nxt, pxp_nxt)
                    if u == TB - 1:
                        emit_xp_chunk(KC, xb_nxt, pxp_nxt)
                        xps_nxt = copy_xp(pxp_nxt)
                        pxp_nxt = None

                # gates: inject xp rows 32u + h @ W_hh, packed PSUM [128, 512]
                pg = pg_pool.tile([128, WQ], F32)
                for q in range(NQ):
                    nc.tensor.matmul(
                        pg[q * B_CORE:(q + 1) * B_CORE, :],
                        idq_t[u * B_CORE:(u + 1) * B_CORE, :],
                        xps_cur[u * B_CORE:(u + 1) * B_CORE,
                                q * WQ:(q + 1) * WQ],
                        start=True, stop=False,
                        tile_position=(u * B_CORE, q * B_CORE),
                        skip_group_check=True)
                for c in range(KH):
                    for q in range(NQ):
                        lhsT = ht[:, c % NTR,
                                  (c // NTR) * B_CORE:(c // NTR + 1) * B_CORE]
                        nc.tensor.matmul(
                            pg[q * B_CORE:(q + 1) * B_CORE, :],
                            lhsT, wc_t[:, KC + c, q * WQ:(q + 1) * WQ],
                            start=False, stop=(c == KH - 1),
                            tile_position=(0, q * B_CORE),
                            skip_group_check=True)

                # activations (packed layout)
                sig = act_pool.tile([128, 3 * HQ], act_dtype, tag="sig")
                nc.scalar.activation(out=sig, in_=pg[:, 0:3 * HQ], func=SIG)
                tg = act_pool.tile([128, HQ], act_dtype, tag="tg")
                nc.scalar.activation(out=tg, in_=pg[:, 3 * HQ:4 * HQ], func=TANH)

                # c_new = sig_f * c + sig_i * tg
                t1 = act_pool.tile([128, HQ], F32, tag="t1")
                nc.vector.tensor_mul(t1, sig[:, HQ:2 * HQ], c_prev)
                t2 = act_pool.tile([128, HQ], F32, tag="t2")
                nc.vector.tensor_mul(t2, sig[:, 0:HQ], tg)
                c_new = state_pool.tile([128, H // NQ], F32, tag="c")
                nc.vector.tensor_add(c_new, t1, t2)

                # h_new = sig_o * tanh(c_new)
                tc_t = act_pool.tile([128, HQ], act_dtype, tag="tc")
                nc.scalar.activation(out=tc_t, in_=c_new, func=TANH)
                hp = act_pool.tile([128, HQ], F32, tag="hp")
                nc.vector.tensor_mul(hp, sig[:, 2 * HQ:3 * HQ], tc_t)

                # write h_new to DRAM output (unpack (q b) j -> b (q j))
                for q in range(NQ):
                    nc.sync.dma_start(
                        out=out_h[s, :, q * HQ:(q + 1) * HQ],
                        in_=hp[q * B_CORE:(q + 1) * B_CORE, :])

                # transpose h_new for the next step's stationary operand
                if ss != total - 1:
                    pt = pt_pool.tile([128, NTR, 128], F32)
                    for t in range(NTR):
                        nc.tensor.transpose(
                            pt[:, t, :], hp[:, t * 128:(t + 1) * 128], id_t)
                    ht = state_pool.tile([128, NTR, 128], mm_dtype, tag="ht")
                    nc.vector.tensor_copy(ht, pt)

                c_prev = c_new
                if u == TB - 1 and xps_nxt is not None:
                    xps_cur = xps_nxt
                    xps_nxt = None

            nc.sync.dma_start(out=c_fin, in_=c_prev)

    nc.compile()
    return nc


def _weights_combined(W_ih, W_hh, b_ih, b_hh):
    """[1025, 2048] combined weights, columns permuted to quarter-blocks
    [i_q | f_q | o_q | g_q] (each HQ wide) for q in range(NQ)."""
    Wc = np.concatenate(
        [W_ih.T, W_hh.T, (b_ih + b_hh)[None, :]], axis=0
    ).astype(np.float32)  # [1025, 2048] in orig col order i|f|g|o
    perm = []
    for q in range(NQ):
        for g in (0, 1, 3, 2):  # i, f, o, g
            perm.extend(range(g * H + q * HQ, g * H + (q + 1) * HQ))
    return np.ascontiguousarray(Wc[:, perm])


def _pack(a):
    """[B_CORE, H] -> packed [128, H//NQ]: row q*B_CORE+b, col j = a[b, HQ*q+j]."""
    return np.ascontiguousarray(
        a.reshape(B_CORE, NQ, HQ).transpose(1, 0, 2).reshape(128, HQ)
    )


def _unpack(p):
    """inverse of _pack"""
    return np.ascontiguousarray(
        p.reshape(NQ, B_CORE, HQ).transpose(1, 0, 2).reshape(B_CORE, H)
    )


def _pack_ht(h):
    """[B_CORE, H] -> [128, NTR, 128]: ht[j, t, q*B_CORE+b] = h[b, HQ*q+128*t+j]"""
    return np.ascontiguousarray(
        h.T.reshape(NQ, NTR, 128, B_CORE).transpose(2, 1, 0, 3).reshape(128, NTR, 128)
    )


def _core_inputs(x_dir, h0, c0, W_ih, W_hh, b_ih, b_hh, bsl, nt):
    """inputs for one core: direction-prepared x [nt,B,I], batch slice bsl."""
    xs = x_dir[:nt, bsl, :]  # [nt, B_CORE, I]
    nb = nt // TB
    xb = np.ascontiguousarray(
        xs.reshape(nb, TB, B_CORE, I).transpose(0, 3, 1, 2).reshape(nb, I, 128))
    wcb = _weights_combined(W_ih, W_hh, b_ih, b_hh)
    return {
        "wc": wcb[:1024].astype(MM_NP),
        "wb": wcb[1024:1025].astype(MM_NP),
        "xblk": xb.astype(MM_NP),
        "h0t": _pack_ht(h0[bsl]).astype(MM_NP),
        "c0p": _pack(c0[bsl]),
        "ident": np.eye(128, dtype=np.float32),
        "identq": np.equal.outer(np.arange(128) % B_CORE,
                                 np.arange(B_CORE)).astype(MM_NP),
        "ones": np.ones((1, 128), MM_NP),
    }


def kernel(x, h0_fwd, c0_fwd, h0_bwd, c0_bwd,
           W_ih_fwd, W_hh_fwd, b_ih_fwd, b_hh_fwd,
           W_ih_bwd, W_hh_bwd, b_ih_bwd, b_hh_bwd):
    x = np.asarray(x, np.float32)
    nt = x.shape[0]
    key = ("nc", nt)
    if key not in _cache:
        _cache[key] = _build(nt)
    nc = _cache[key]

    x_rev = x[::-1]
    in_maps = []
    specs = [
        (x, h0_fwd, c0_fwd, W_ih_fwd, W_hh_fwd, b_ih_fwd, b_hh_fwd, slice(0, 32)),
        (x, h0_fwd, c0_fwd, W_ih_fwd, W_hh_fwd, b_ih_fwd, b_hh_fwd, slice(32, 64)),
        (x_rev, h0_bwd, c0_bwd, W_ih_bwd, W_hh_bwd, b_ih_bwd, b_hh_bwd, slice(0, 32)),
        (x_rev, h0_bwd, c0_bwd, W_ih_bwd, W_hh_bwd, b_ih_bwd, b_hh_bwd, slice(32, 64)),
    ]
    for xd, h0, c0, Wi, Wh, bi, bh, bsl in specs:
        in_maps.append(_core_inputs(
            np.asarray(xd), np.asarray(h0), np.asarray(c0),
            np.asarray(Wi), np.asarray(Wh), np.asarray(bi), np.asarray(bh),
            bsl, nt))
    # cores 4-7: duplicates of 0-3 (outputs ignored)
    in_maps = in_maps + [dict(m) for m in in_maps]

    res = run_bass_kernel_spmd(nc, in_maps, core_ids=list(range(8)))
    kernel.last_exec_time_ns = res.exec_time_ns
    r = res.results

    out = np.empty((nt, B, 2 * H), np.float32)
    out[:, 0:32, 0:H] = r[0]["out_h"]
    out[:, 32:64, 0:H] = r[1]["out_h"]
    out[:, 0:32, H:2 * H] = r[2]["out_h"][::-1]
    out[:, 32:64, H:2 * H] = r[3]["out_h"][::-1]
    hT_f = out[nt - 1, :, 0:H].copy()
    hT_b = out[0, :, H:2 * H].copy()
    cT_f = np.concatenate([_unpack(r[0]["c_fin"]), _unpack(r[1]["c_fin"])], 0)
    cT_b = np.concatenate([_unpack(r[2]["c_fin"]), _unpack(r[3]["c_fin"])], 0)
    return out, hT_f, cT_f, hT_b, cT_b


kernel.last_exec_time_ns = None
